# revision 29
# baseline (speedup 1.0000x reference)
"""Trainium2 Bass kernel for nn_DecoderRNN (B=8192, H=1024, IN=OUT=4).

Data-parallel over 8 NeuronCores: batch shard of 1024 rows per core, all
weights replicated. Per core, everything stays SBUF-resident:

  ec   = relu(context @ wc.T + bc)                      [B, 512]
  gi_g = ec @ wi_g.T + bi_g   (precomputed per GRU)     [B, 3072]
  90 GRU steps (60 + 30), each:
      gh = h @ wh.T            (fp16 matmul, fp32 PSUM accumulation)
      r  = sigmoid(gh_r + bh_r + gi_r)
      z  = sigmoid(gh_z + bh_z + gi_z)
      n  = tanh((gh_n + bh_n) * r + gi_n)
      h  = (1-z)*n + z*h                                (fp32, in place)
      fc projection of h (resp. relu(h)) -> [4, B] -> DRAM

Layouts are transposed on host so the hidden dim is the partition dim:
h^T is [H, B] = 8 chunks of [128, B]; gh^T is [3H, B] = 24 m-tiles.
Matmul inputs are fp16 (same PE rate as bf16, 3 extra mantissa bits;
weights rounded once on host, h re-rounded each step); all accumulation,
state and gate math is fp32. Measured ~9.1-9.3 ms on HW (PE ~99% busy,
clock-throttle limited), absmax rel err ~3e-4 vs the fp32 reference.
"""

import os
import sys

sys.path.insert(0, "/opt/trn_rl_repo")

import numpy as np

import concourse.mybir as mybir
import concourse.tile as tile
from concourse import bacc
from concourse.bass_utils import run_bass_kernel_spmd

N_CORES = 8
B_FULL, H, IN, OUT = 8192, 1024, 4, 4
B = B_FULL // N_CORES          # batch rows per core
P = 128                        # partitions
KT = H // P                    # h chunks (8)
GT = 3 * H // P                # gate m-tiles (24)
C = H // 2                     # encoded-context dim (512)
CT = C // P                    # ec chunks (4)
NH = 512                       # PSUM half width (bank limit for fp32)
F32 = mybir.dt.float32
BF16 = mybir.dt.float16      # matmul operand dtype
NP_MM = np.float16

_BUILD_CACHE = {}
DEBUG_DUMPS = bool(int(os.environ.get("DECODER_DEBUG", "0")))


def _build(T1, T2):
    nc = bacc.Bacc("TRN2", target_bir_lowering=False, debug=False,
                   num_devices=N_CORES)
    dbg = {}
    if DEBUG_DUMPS:
        dbg["ec"] = nc.dram_tensor("dbg_ec", [P, CT, B], BF16, kind="ExternalOutput")
        dbg["gi1"] = nc.dram_tensor("dbg_gi1", [P, GT, B], BF16, kind="ExternalOutput")
        dbg["h1"] = nc.dram_tensor("dbg_h1", [P, KT, B], F32, kind="ExternalOutput")
        for nm in ("r", "z", "n", "ghr", "prer"):
            dbg[nm] = nc.dram_tensor(f"dbg_{nm}", [P, KT, B], F32,
                                     kind="ExternalOutput")

    ctx32 = nc.dram_tensor("ctx32", [H, B], F32, kind="ExternalInput")
    ctxbf = nc.dram_tensor("ctxbf", [H, B], BF16, kind="ExternalInput")
    wcT = nc.dram_tensor("wcT", [H, C], BF16, kind="ExternalInput")
    bc = nc.dram_tensor("bc", [P, CT], F32, kind="ExternalInput")
    phases = []
    for g, T, relu_fc in ((1, T1, False), (2, T2, True)):
        wiT = nc.dram_tensor(f"wiT{g}", [C, 3 * H], BF16, kind="ExternalInput")
        whT = nc.dram_tensor(f"whT{g}", [H, 3 * H], BF16, kind="ExternalInput")
        bi = nc.dram_tensor(f"bi{g}", [P, GT], F32, kind="ExternalInput")
        bh = nc.dram_tensor(f"bh{g}", [P, GT], F32, kind="ExternalInput")
        fcT = nc.dram_tensor(f"fcT{g}", [H, 4], BF16, kind="ExternalInput")
        fcb = nc.dram_tensor(f"fcb{g}", [4, 1], F32, kind="ExternalInput")
        out_d = nc.dram_tensor(f"proj{g}", [T, 4, B], F32, kind="ExternalOutput")
        phases.append((g, T, relu_fc, wiT, whT, bi, bh, fcT, fcb, out_d))

    with tile.TileContext(nc) as tc:
        with tc.tile_pool(name="const", bufs=1) as const, \
             tc.tile_pool(name="state", bufs=1) as state, \
             tc.tile_pool(name="work", bufs=2) as work, \
             tc.tile_pool(name="projp", bufs=1) as projp, \
             tc.tile_pool(name="wij", bufs=2) as wij, \
             tc.tile_pool(name="psg", bufs=6, space="PSUM") as psg, \
             tc.tile_pool(name="psf", bufs=1, space="PSUM") as psf:

            # ---- resident constants -------------------------------------
            wc_sb = const.tile([P, KT, C], BF16, tag="wc")
            nc.sync.dma_start(out=wc_sb, in_=wcT.rearrange("(k p) c -> p k c", p=P))
            bc_sb = const.tile([P, CT], F32, tag="bc")
            nc.sync.dma_start(out=bc_sb, in_=bc[:, :])

            bias_sb = {}
            fc_sb = {}
            for (g, T, relu_fc, wiT, whT, bi, bh, fcT, fcb, out_d) in phases:
                bi_sb = const.tile([P, GT], F32, tag=f"bi{g}")
                nc.sync.dma_start(out=bi_sb, in_=bi[:, :])
                bh_sb = const.tile([P, GT], F32, tag=f"bh{g}")
                nc.sync.dma_start(out=bh_sb, in_=bh[:, :])
                f_sb = const.tile([P, KT, 4], BF16, tag=f"fc{g}")
                nc.sync.dma_start(out=f_sb, in_=fcT.rearrange("(k p) i -> p k i", p=P))
                fb_sb = const.tile([4, 1], F32, tag=f"fcb{g}")
                nc.sync.dma_start(out=fb_sb, in_=fcb[:, :])
                bias_sb[g] = (bi_sb, bh_sb)
                fc_sb[g] = (f_sb, fb_sb)

            # ---- state tiles --------------------------------------------
            h32 = state.tile([P, KT, B], F32, tag="h32")
            hbf = state.tile([P, KT, B], BF16, tag="hbf")
            rh = state.tile([P, KT, B], BF16, tag="rh")     # relu(h) for fc2
            ec_sb = state.tile([P, CT, B], BF16, tag="ec")
            gi_sb = state.tile([P, GT, B], BF16, tag="gi")
            wh_sb = state.tile([P, KT, 3 * H], BF16, tag="wh")

            nc.sync.dma_start(out=h32, in_=ctx32.rearrange("(k p) b -> p k b", p=P))
            nc.sync.dma_start(out=hbf, in_=ctxbf.rearrange("(k p) b -> p k b", p=P))

            # ---- ec = relu(wc @ ctx + bc), fp16 -------------------------
            for m in range(CT):
                for hf in range(2):
                    sl = slice(hf * NH, (hf + 1) * NH)
                    pt = psg.tile([P, NH], F32, tag="gh")
                    for k in range(KT):
                        nc.tensor.matmul(pt[:, :],
                                         wc_sb[:, k, m * P:(m + 1) * P],
                                         hbf[:, k, sl],
                                         start=(k == 0), stop=(k == KT - 1))
                    nc.scalar.activation(out=ec_sb[:, m, sl], in_=pt[:, :],
                                         func=mybir.ActivationFunctionType.Relu,
                                         bias=bc_sb[:, m:m + 1])
            if DEBUG_DUMPS:
                nc.sync.dma_start(out=dbg["ec"][:, :, :], in_=ec_sb[:, :, :])

            # ---- phases -------------------------------------------------
            for (g, T, relu_fc, wiT, whT, bi, bh, fcT, fcb, out_d) in phases:
                bi_sb, bh_sb = bias_sb[g]
                wiT_r = wiT.rearrange("(k p) t -> p k t", p=P)

                # gi = wi @ ec + bi  -> fp16, [P, GT, B]
                for m in range(GT):
                    wi_m = wij.tile([P, CT, P], BF16, tag="wim")
                    nc.sync.dma_start(out=wi_m,
                                      in_=wiT_r[:, :, m * P:(m + 1) * P])
                    for hf in range(2):
                        sl = slice(hf * NH, (hf + 1) * NH)
                        pt = psg.tile([P, NH], F32, tag="gh")
                        for k in range(CT):
                            nc.tensor.matmul(pt[:, :], wi_m[:, k, :],
                                             ec_sb[:, k, sl],
                                             start=(k == 0), stop=(k == CT - 1))
                        nc.scalar.activation(out=gi_sb[:, m, sl], in_=pt[:, :],
                                             func=mybir.ActivationFunctionType.Identity,
                                             bias=bi_sb[:, m:m + 1])

                if DEBUG_DUMPS and g == 1:
                    nc.sync.dma_start(out=dbg["gi1"][:, :, :], in_=gi_sb[:, :, :])
                # wh resident for this phase
                nc.sync.dma_start(out=wh_sb,
                                  in_=whT.rearrange("(k p) t -> p k t", p=P))
                if g == 2:
                    # restart recurrence from the context
                    nc.sync.dma_start(out=h32,
                                      in_=ctx32.rearrange("(k p) b -> p k b", p=P))
                    nc.sync.dma_start(out=hbf,
                                      in_=ctxbf.rearrange("(k p) b -> p k b", p=P))

                f_sb, fb_sb = fc_sb[g]
                for t in range(T):
                    for c in range(KT):
                        pts = []
                        for m in (c, KT + c, 2 * KT + c):   # r, z, n tiles
                            halves = []
                            for hf in range(2):
                                ph = psg.tile([P, NH], F32, tag="gh")
                                sl = slice(hf * NH, (hf + 1) * NH)
                                for k in range(KT):
                                    nc.tensor.matmul(
                                        ph[:, :],
                                        wh_sb[:, k, m * P:(m + 1) * P],
                                        hbf[:, k, sl],
                                        start=(k == 0), stop=(k == KT - 1))
                                halves.append(ph)
                            pts.append(halves)
                        pt_r, pt_z, pt_n = pts
                        A = mybir.AluOpType
                        if DEBUG_DUMPS and g == 1 and t == 0:
                            ghd = projp.tile([P, B], F32, tag="proj")
                            for hf in range(2):
                                nc.scalar.activation(
                                    out=ghd[:, hf * NH:(hf + 1) * NH],
                                    in_=pt_r[hf][:, :],
                                    func=mybir.ActivationFunctionType.Copy)
                            nc.sync.dma_start(out=dbg["ghr"][:, c, :], in_=ghd[:, :])
                        # r = sigmoid(gh_r + bh_r + gi_r); STT drains PSUM to
                        # SBUF in one pass, activation runs in place on SBUF
                        r_sb = work.tile([P, B], F32, tag="r")
                        for hf in range(2):
                            sl = slice(hf * NH, (hf + 1) * NH)
                            nc.vector.scalar_tensor_tensor(
                                out=r_sb[:, sl], in0=pt_r[hf][:, :],
                                scalar=bh_sb[:, c:c + 1], in1=gi_sb[:, c, sl],
                                op0=A.add, op1=A.add)
                        if DEBUG_DUMPS and g == 1 and t == 0:
                            pred = projp.tile([P, B], F32, tag="proj")
                            nc.scalar.activation(
                                out=pred[:, :], in_=r_sb[:, :],
                                func=mybir.ActivationFunctionType.Copy)
                            nc.sync.dma_start(out=dbg["prer"][:, c, :], in_=pred[:, :])
                        nc.scalar.activation(out=r_sb[:, :], in_=r_sb[:, :],
                                             func=mybir.ActivationFunctionType.Sigmoid)
                        # z = sigmoid(gh_z + bh_z + gi_z)
                        z_sb = work.tile([P, B], F32, tag="z")
                        for hf in range(2):
                            sl = slice(hf * NH, (hf + 1) * NH)
                            nc.vector.scalar_tensor_tensor(
                                out=z_sb[:, sl], in0=pt_z[hf][:, :],
                                scalar=bh_sb[:, KT + c:KT + c + 1],
                                in1=gi_sb[:, KT + c, sl], op0=A.add, op1=A.add)
                        nc.scalar.activation(out=z_sb[:, :], in_=z_sb[:, :],
                                             func=mybir.ActivationFunctionType.Sigmoid)
                        # n = tanh((gh_n + bh_n)*r + gi_n)
                        n_sb = work.tile([P, B], F32, tag="n")
                        for hf in range(2):
                            sl = slice(hf * NH, (hf + 1) * NH)
                            nc.vector.scalar_tensor_tensor(
                                out=n_sb[:, sl], in0=pt_n[hf][:, :],
                                scalar=bh_sb[:, 2 * KT + c:2 * KT + c + 1],
                                in1=r_sb[:, sl], op0=A.add, op1=A.mult)
                        nc.vector.tensor_add(n_sb[:, :], n_sb[:, :],
                                             gi_sb[:, 2 * KT + c, :])
                        nc.scalar.activation(out=n_sb[:, :], in_=n_sb[:, :],
                                             func=mybir.ActivationFunctionType.Tanh)
                        if DEBUG_DUMPS and g == 1 and t == 0:
                            nc.sync.dma_start(out=dbg["r"][:, c, :], in_=r_sb[:, :])
                            nc.sync.dma_start(out=dbg["z"][:, c, :], in_=z_sb[:, :])
                            nc.sync.dma_start(out=dbg["n"][:, c, :], in_=n_sb[:, :])
                        # h = (1-z)*n + z*h  ==  ((h - n) * z) + n, in place
                        hc = h32[:, c, :]
                        nc.vector.tensor_sub(hc, hc, n_sb[:, :])
                        nc.vector.tensor_mul(hc, hc, z_sb[:, :])
                        nc.vector.tensor_add(hc, hc, n_sb[:, :])
                    # fp16 copies only after ALL of this step's matmuls have
                    # read the old h (hbf is updated in place). GpSimd is
                    # otherwise idle, so the casts don't queue behind the
                    # last chunks' sigmoids/tanh on ScalarE.
                    for c in range(KT):
                        nc.gpsimd.tensor_copy(out=hbf[:, c, :], in_=h32[:, c, :])
                        if relu_fc:
                            nc.scalar.activation(out=rh[:, c, :], in_=h32[:, c, :],
                                                 func=mybir.ActivationFunctionType.Relu)
                    # fc projection of this step's h
                    src = rh if relu_fc else hbf
                    ptf = psf.tile([4, B], F32, tag="fc")
                    for hf in range(2):
                        sl = slice(hf * NH, (hf + 1) * NH)
                        for k in range(KT):
                            nc.tensor.matmul(ptf[:, sl], f_sb[:, k, :],
                                             src[:, k, sl],
                                             start=(k == 0), stop=(k == KT - 1))
                    proj = projp.tile([4, B], F32, tag="proj")
                    nc.scalar.activation(out=proj[:, :], in_=ptf[:, :],
                                         func=mybir.ActivationFunctionType.Identity,
                                         bias=fb_sb[:, 0:1])
                    nc.sync.dma_start(out=out_d[t], in_=proj[:, :])
                    if DEBUG_DUMPS and g == 1 and t == 0:
                        nc.sync.dma_start(out=dbg["h1"][:, :, :], in_=h32[:, :, :])
    nc.compile()
    return nc


def _get_nc(T1, T2):
    key = (T1, T2)
    if key not in _BUILD_CACHE:
        _BUILD_CACHE[key] = _build(T1, T2)
    return _BUILD_CACHE[key]


def kernel(context, wc, bc,
           gru1_wi, gru1_wh, gru1_bi, gru1_bh,
           gru2_wi, gru2_wh, gru2_bi, gru2_bh,
           fc_in_w, fc_in_b, fc_out_w, fc_out_b,
           future_length, past_length, _trace=False):
    T1, T2 = int(past_length), int(future_length)
    context = np.asarray(context, np.float32)

    def pT(w):     # [r, c] -> bf16 transposed [c, r]
        return np.ascontiguousarray(np.asarray(w, np.float32).T).astype(NP_MM)

    def chunked(b):   # [n*128] -> [128, n]
        v = np.asarray(b, np.float32)
        return np.ascontiguousarray(v.reshape(-1, P).T)

    shared = {
        "wcT": pT(wc), "bc": chunked(bc),
        "wiT1": pT(gru1_wi), "whT1": pT(gru1_wh),
        "bi1": chunked(gru1_bi), "bh1": chunked(gru1_bh), "fcT1": pT(fc_in_w),
        "wiT2": pT(gru2_wi), "whT2": pT(gru2_wh),
        "bi2": chunked(gru2_bi), "bh2": chunked(gru2_bh), "fcT2": pT(fc_out_w),
        "fcb1": np.asarray(fc_in_b, np.float32).reshape(4, 1).copy(),
        "fcb2": np.asarray(fc_out_b, np.float32).reshape(4, 1).copy(),
    }
    in_maps = []
    for cix in range(N_CORES):
        shard = context[cix * B:(cix + 1) * B]               # [B, H]
        ctxT = np.ascontiguousarray(shard.T)                 # [H, B]
        m = dict(shared)
        m["ctx32"] = ctxT
        m["ctxbf"] = ctxT.astype(NP_MM)
        in_maps.append(m)

    nc = _get_nc(T1, T2)
    res = run_bass_kernel_spmd(nc, in_maps, list(range(N_CORES)),
                               trace=_trace)
    kernel.last_results = res

    dec = np.empty((B_FULL, T1, IN), np.float32)
    out = np.empty((B_FULL, T2, OUT), np.float32)
    for cix in range(N_CORES):
        dec[cix * B:(cix + 1) * B] = res.results[cix]["proj1"].transpose(2, 0, 1)
        out[cix * B:(cix + 1) * B] = res.results[cix]["proj2"].transpose(2, 0, 1)
    return dec, out


# revision 30
# speedup vs baseline: 1.1449x; 1.1449x over previous
"""Trainium2 Bass kernel for nn_DecoderRNN (B=8192, H=1024, IN=OUT=4).

Data-parallel over 8 NeuronCores: batch shard of 1024 rows per core, all
weights replicated. Per core, everything stays SBUF-resident:

  ec   = relu(context @ wc.T + bc)                      [B, 512]
  gi_g = ec @ wi_g.T + bi_g   (precomputed per GRU)     [B, 3072]
  90 GRU steps (60 + 30), each:
      gh = h @ wh.T            (fp16 matmul, fp32 PSUM accumulation)
      r  = sigmoid(gh_r + bh_r + gi_r)
      z  = sigmoid(gh_z + bh_z + gi_z)
      n  = tanh((gh_n + bh_n) * r + gi_n)
      h  = (1-z)*n + z*h                                (fp32, in place)
      fc projection of h (resp. relu(h)) -> [4, B] -> DRAM

Layouts are transposed on host so the hidden dim is the partition dim:
h^T is [H, B] = 8 chunks of [128, B]; gh^T is [3H, B] = 24 m-tiles.
Matmul inputs are fp16 (same PE rate as bf16, 3 extra mantissa bits;
weights rounded once on host, h re-rounded each step); all accumulation,
state and gate math is fp32. Measured ~9.1-9.3 ms on HW (PE ~99% busy,
clock-throttle limited), absmax rel err ~3e-4 vs the fp32 reference.
"""

import os
import sys

sys.path.insert(0, "/opt/trn_rl_repo")

import numpy as np

import concourse.mybir as mybir
import concourse.tile as tile
from concourse import bacc
from concourse.bass_utils import run_bass_kernel_spmd

N_CORES = 8
B_FULL, H, IN, OUT = 8192, 1024, 4, 4
B = B_FULL // N_CORES          # batch rows per core
P = 128                        # partitions
KT = H // P                    # h chunks (8)
GT = 3 * H // P                # gate m-tiles (24)
C = H // 2                     # encoded-context dim (512)
CT = C // P                    # ec chunks (4)
NH = 512                       # PSUM half width (bank limit for fp32)
F32 = mybir.dt.float32
BF16 = mybir.dt.float16      # matmul operand dtype
NP_MM = np.float16

_BUILD_CACHE = {}
DEBUG_DUMPS = bool(int(os.environ.get("DECODER_DEBUG", "0")))


def _build(T1, T2):
    nc = bacc.Bacc("TRN2", target_bir_lowering=False, debug=False,
                   num_devices=N_CORES)
    dbg = {}
    if DEBUG_DUMPS:
        dbg["ec"] = nc.dram_tensor("dbg_ec", [P, CT, B], BF16, kind="ExternalOutput")
        dbg["gi1"] = nc.dram_tensor("dbg_gi1", [P, GT, B], BF16, kind="ExternalOutput")
        dbg["h1"] = nc.dram_tensor("dbg_h1", [P, KT, B], F32, kind="ExternalOutput")
        for nm in ("r", "z", "n", "ghr", "prer"):
            dbg[nm] = nc.dram_tensor(f"dbg_{nm}", [P, KT, B], F32,
                                     kind="ExternalOutput")

    ctx32 = nc.dram_tensor("ctx32", [H, B], F32, kind="ExternalInput")
    ctxbf = nc.dram_tensor("ctxbf", [H, B], BF16, kind="ExternalInput")
    wcT = nc.dram_tensor("wcT", [H, C], BF16, kind="ExternalInput")
    bc = nc.dram_tensor("bc", [P, CT], F32, kind="ExternalInput")
    phases = []
    for g, T, relu_fc in ((1, T1, False), (2, T2, True)):
        wiT = nc.dram_tensor(f"wiT{g}", [C, 3 * H], BF16, kind="ExternalInput")
        whT = nc.dram_tensor(f"whT{g}", [H, 3 * H], BF16, kind="ExternalInput")
        bi = nc.dram_tensor(f"bi{g}", [P, GT], F32, kind="ExternalInput")
        bh = nc.dram_tensor(f"bh{g}", [P, GT], F32, kind="ExternalInput")
        fcT = nc.dram_tensor(f"fcT{g}", [H, 4], BF16, kind="ExternalInput")
        fcb = nc.dram_tensor(f"fcb{g}", [4, 1], F32, kind="ExternalInput")
        out_d = nc.dram_tensor(f"proj{g}", [T, 4, B], F32, kind="ExternalOutput")
        phases.append((g, T, relu_fc, wiT, whT, bi, bh, fcT, fcb, out_d))

    with tile.TileContext(nc) as tc:
        with tc.tile_pool(name="const", bufs=1) as const, \
             tc.tile_pool(name="state", bufs=1) as state, \
             tc.tile_pool(name="work", bufs=2) as work, \
             tc.tile_pool(name="projp", bufs=1) as projp, \
             tc.tile_pool(name="wij", bufs=2) as wij, \
             tc.tile_pool(name="psg", bufs=6, space="PSUM") as psg, \
             tc.tile_pool(name="psf", bufs=1, space="PSUM") as psf:

            # ---- resident constants -------------------------------------
            wc_sb = const.tile([P, KT, C], BF16, tag="wc")
            nc.sync.dma_start(out=wc_sb, in_=wcT.rearrange("(k p) c -> p k c", p=P))
            bc_sb = const.tile([P, CT], F32, tag="bc")
            nc.sync.dma_start(out=bc_sb, in_=bc[:, :])

            bias_sb = {}
            fc_sb = {}
            for (g, T, relu_fc, wiT, whT, bi, bh, fcT, fcb, out_d) in phases:
                bi_sb = const.tile([P, GT], F32, tag=f"bi{g}")
                nc.sync.dma_start(out=bi_sb, in_=bi[:, :])
                bh_sb = const.tile([P, GT], F32, tag=f"bh{g}")
                nc.sync.dma_start(out=bh_sb, in_=bh[:, :])
                f_sb = const.tile([P, KT, 4], BF16, tag=f"fc{g}")
                nc.sync.dma_start(out=f_sb, in_=fcT.rearrange("(k p) i -> p k i", p=P))
                fb_sb = const.tile([4, 1], F32, tag=f"fcb{g}")
                nc.sync.dma_start(out=fb_sb, in_=fcb[:, :])
                bias_sb[g] = (bi_sb, bh_sb)
                fc_sb[g] = (f_sb, fb_sb)

            # ---- state tiles --------------------------------------------
            h32 = state.tile([P, KT, B], F32, tag="h32")
            hbf = state.tile([P, KT, B], BF16, tag="hbf")
            rh = state.tile([P, KT, B], BF16, tag="rh")     # relu(h) for fc2
            ec_sb = state.tile([P, CT, B], BF16, tag="ec")
            gi_sb = state.tile([P, GT, B], BF16, tag="gi")
            wh_sb = state.tile([P, KT, 3 * H], BF16, tag="wh")

            nc.sync.dma_start(out=h32, in_=ctx32.rearrange("(k p) b -> p k b", p=P))
            nc.sync.dma_start(out=hbf, in_=ctxbf.rearrange("(k p) b -> p k b", p=P))

            # ---- ec = relu(wc @ ctx + bc), fp16 -------------------------
            for m in range(CT):
                for hf in range(2):
                    sl = slice(hf * NH, (hf + 1) * NH)
                    pt = psg.tile([P, NH], F32, tag="gh")
                    for k in range(KT):
                        nc.tensor.matmul(pt[:, :],
                                         wc_sb[:, k, m * P:(m + 1) * P],
                                         hbf[:, k, sl],
                                         start=(k == 0), stop=(k == KT - 1))
                    nc.scalar.activation(out=ec_sb[:, m, sl], in_=pt[:, :],
                                         func=mybir.ActivationFunctionType.Relu,
                                         bias=bc_sb[:, m:m + 1])
            if DEBUG_DUMPS:
                nc.sync.dma_start(out=dbg["ec"][:, :, :], in_=ec_sb[:, :, :])

            # ---- phases -------------------------------------------------
            for (g, T, relu_fc, wiT, whT, bi, bh, fcT, fcb, out_d) in phases:
                bi_sb, bh_sb = bias_sb[g]
                wiT_r = wiT.rearrange("(k p) t -> p k t", p=P)

                # gi = wi @ ec + bi  -> fp16, [P, GT, B]
                for m in range(GT):
                    wi_m = wij.tile([P, CT, P], BF16, tag="wim")
                    nc.sync.dma_start(out=wi_m,
                                      in_=wiT_r[:, :, m * P:(m + 1) * P])
                    for hf in range(2):
                        sl = slice(hf * NH, (hf + 1) * NH)
                        pt = psg.tile([P, NH], F32, tag="gh")
                        for k in range(CT):
                            nc.tensor.matmul(pt[:, :], wi_m[:, k, :],
                                             ec_sb[:, k, sl],
                                             start=(k == 0), stop=(k == CT - 1))
                        nc.scalar.activation(out=gi_sb[:, m, sl], in_=pt[:, :],
                                             func=mybir.ActivationFunctionType.Identity,
                                             bias=bi_sb[:, m:m + 1])

                if DEBUG_DUMPS and g == 1:
                    nc.sync.dma_start(out=dbg["gi1"][:, :, :], in_=gi_sb[:, :, :])
                # wh resident for this phase
                nc.sync.dma_start(out=wh_sb,
                                  in_=whT.rearrange("(k p) t -> p k t", p=P))
                if g == 2:
                    # restart recurrence from the context
                    nc.sync.dma_start(out=h32,
                                      in_=ctx32.rearrange("(k p) b -> p k b", p=P))
                    nc.sync.dma_start(out=hbf,
                                      in_=ctxbf.rearrange("(k p) b -> p k b", p=P))

                f_sb, fb_sb = fc_sb[g]
                for t in range(T):
                    for c in range(KT):
                        pts = []
                        for m in (c, KT + c, 2 * KT + c):   # r, z, n tiles
                            halves = []
                            for hf in range(2):
                                ph = psg.tile([P, NH], F32, tag="gh")
                                sl = slice(hf * NH, (hf + 1) * NH)
                                for k in range(KT):
                                    nc.tensor.matmul(
                                        ph[:, :],
                                        wh_sb[:, k, m * P:(m + 1) * P],
                                        hbf[:, k, sl],
                                        start=(k == 0), stop=(k == KT - 1))
                                halves.append(ph)
                            pts.append(halves)
                        pt_r, pt_z, pt_n = pts
                        A = mybir.AluOpType
                        if DEBUG_DUMPS and g == 1 and t == 0:
                            ghd = projp.tile([P, B], F32, tag="proj")
                            for hf in range(2):
                                nc.scalar.activation(
                                    out=ghd[:, hf * NH:(hf + 1) * NH],
                                    in_=pt_r[hf][:, :],
                                    func=mybir.ActivationFunctionType.Copy)
                            nc.sync.dma_start(out=dbg["ghr"][:, c, :], in_=ghd[:, :])
                        # r = sigmoid(gh_r + bh_r + gi_r); STT drains PSUM to
                        # SBUF in one pass, activation runs in place on SBUF
                        r_sb = work.tile([P, B], F32, tag="r")
                        for hf in range(2):
                            sl = slice(hf * NH, (hf + 1) * NH)
                            nc.vector.scalar_tensor_tensor(
                                out=r_sb[:, sl], in0=pt_r[hf][:, :],
                                scalar=bh_sb[:, c:c + 1], in1=gi_sb[:, c, sl],
                                op0=A.add, op1=A.add)
                        if DEBUG_DUMPS and g == 1 and t == 0:
                            pred = projp.tile([P, B], F32, tag="proj")
                            nc.scalar.activation(
                                out=pred[:, :], in_=r_sb[:, :],
                                func=mybir.ActivationFunctionType.Copy)
                            nc.sync.dma_start(out=dbg["prer"][:, c, :], in_=pred[:, :])
                        nc.scalar.activation(out=r_sb[:, :], in_=r_sb[:, :],
                                             func=mybir.ActivationFunctionType.Sigmoid)
                        # z = sigmoid(gh_z + bh_z + gi_z)
                        z_sb = work.tile([P, B], F32, tag="z")
                        for hf in range(2):
                            sl = slice(hf * NH, (hf + 1) * NH)
                            nc.vector.scalar_tensor_tensor(
                                out=z_sb[:, sl], in0=pt_z[hf][:, :],
                                scalar=bh_sb[:, KT + c:KT + c + 1],
                                in1=gi_sb[:, KT + c, sl], op0=A.add, op1=A.add)
                        nc.scalar.activation(out=z_sb[:, :], in_=z_sb[:, :],
                                             func=mybir.ActivationFunctionType.Sigmoid)
                        # n = tanh((gh_n + bh_n)*r + gi_n)
                        n_sb = work.tile([P, B], F32, tag="n")
                        for hf in range(2):
                            sl = slice(hf * NH, (hf + 1) * NH)
                            nc.vector.scalar_tensor_tensor(
                                out=n_sb[:, sl], in0=pt_n[hf][:, :],
                                scalar=bh_sb[:, 2 * KT + c:2 * KT + c + 1],
                                in1=r_sb[:, sl], op0=A.add, op1=A.mult)
                        nc.vector.tensor_add(n_sb[:, :], n_sb[:, :],
                                             gi_sb[:, 2 * KT + c, :])
                        nc.scalar.activation(out=n_sb[:, :], in_=n_sb[:, :],
                                             func=mybir.ActivationFunctionType.Tanh)
                        if DEBUG_DUMPS and g == 1 and t == 0:
                            nc.sync.dma_start(out=dbg["r"][:, c, :], in_=r_sb[:, :])
                            nc.sync.dma_start(out=dbg["z"][:, c, :], in_=z_sb[:, :])
                            nc.sync.dma_start(out=dbg["n"][:, c, :], in_=n_sb[:, :])
                        # h = (1-z)*n + z*h  ==  ((h - n) * z) + n, in place
                        hc = h32[:, c, :]
                        nc.vector.tensor_sub(hc, hc, n_sb[:, :])
                        nc.vector.tensor_mul(hc, hc, z_sb[:, :])
                        nc.vector.tensor_add(hc, hc, n_sb[:, :])
                    # fp16 copies only after ALL of this step's matmuls have
                    # read the old h (hbf is updated in place)
                    for c in range(KT):
                        nc.scalar.activation(out=hbf[:, c, :], in_=h32[:, c, :],
                                             func=mybir.ActivationFunctionType.Copy)
                        if relu_fc:
                            nc.scalar.activation(out=rh[:, c, :], in_=h32[:, c, :],
                                                 func=mybir.ActivationFunctionType.Relu)
                    # fc projection of this step's h
                    src = rh if relu_fc else hbf
                    ptf = psf.tile([4, B], F32, tag="fc")
                    for hf in range(2):
                        sl = slice(hf * NH, (hf + 1) * NH)
                        for k in range(KT):
                            nc.tensor.matmul(ptf[:, sl], f_sb[:, k, :],
                                             src[:, k, sl],
                                             start=(k == 0), stop=(k == KT - 1))
                    proj = projp.tile([4, B], F32, tag="proj")
                    nc.scalar.activation(out=proj[:, :], in_=ptf[:, :],
                                         func=mybir.ActivationFunctionType.Identity,
                                         bias=fb_sb[:, 0:1])
                    nc.sync.dma_start(out=out_d[t], in_=proj[:, :])
                    if DEBUG_DUMPS and g == 1 and t == 0:
                        nc.sync.dma_start(out=dbg["h1"][:, :, :], in_=h32[:, :, :])
    nc.compile()
    return nc


def _get_nc(T1, T2):
    key = (T1, T2)
    if key not in _BUILD_CACHE:
        _BUILD_CACHE[key] = _build(T1, T2)
    return _BUILD_CACHE[key]


def kernel(context, wc, bc,
           gru1_wi, gru1_wh, gru1_bi, gru1_bh,
           gru2_wi, gru2_wh, gru2_bi, gru2_bh,
           fc_in_w, fc_in_b, fc_out_w, fc_out_b,
           future_length, past_length, _trace=False):
    T1, T2 = int(past_length), int(future_length)
    context = np.asarray(context, np.float32)

    def pT(w):     # [r, c] -> bf16 transposed [c, r]
        return np.ascontiguousarray(np.asarray(w, np.float32).T).astype(NP_MM)

    def chunked(b):   # [n*128] -> [128, n]
        v = np.asarray(b, np.float32)
        return np.ascontiguousarray(v.reshape(-1, P).T)

    shared = {
        "wcT": pT(wc), "bc": chunked(bc),
        "wiT1": pT(gru1_wi), "whT1": pT(gru1_wh),
        "bi1": chunked(gru1_bi), "bh1": chunked(gru1_bh), "fcT1": pT(fc_in_w),
        "wiT2": pT(gru2_wi), "whT2": pT(gru2_wh),
        "bi2": chunked(gru2_bi), "bh2": chunked(gru2_bh), "fcT2": pT(fc_out_w),
        "fcb1": np.asarray(fc_in_b, np.float32).reshape(4, 1).copy(),
        "fcb2": np.asarray(fc_out_b, np.float32).reshape(4, 1).copy(),
    }
    in_maps = []
    for cix in range(N_CORES):
        shard = context[cix * B:(cix + 1) * B]               # [B, H]
        ctxT = np.ascontiguousarray(shard.T)                 # [H, B]
        m = dict(shared)
        m["ctx32"] = ctxT
        m["ctxbf"] = ctxT.astype(NP_MM)
        in_maps.append(m)

    nc = _get_nc(T1, T2)
    res = run_bass_kernel_spmd(nc, in_maps, list(range(N_CORES)),
                               trace=_trace)
    kernel.last_results = res

    dec = np.empty((B_FULL, T1, IN), np.float32)
    out = np.empty((B_FULL, T2, OUT), np.float32)
    for cix in range(N_CORES):
        dec[cix * B:(cix + 1) * B] = res.results[cix]["proj1"].transpose(2, 0, 1)
        out[cix * B:(cix + 1) * B] = res.results[cix]["proj2"].transpose(2, 0, 1)
    return dec, out


# revision 31
# speedup vs baseline: 1.3371x; 1.1678x over previous
"""Trainium2 Bass kernel for nn_DecoderRNN (B=8192, H=1024, IN=OUT=4).

Data-parallel over 8 NeuronCores: batch shard of 1024 rows per core, all
weights replicated. Per core, everything stays SBUF-resident:

  ec   = relu(context @ wc.T + bc)                      [B, 512]
  gi_g = ec @ wi_g.T + bi_g   (precomputed per GRU)     [B, 3072]
  90 GRU steps (60 + 30), each:
      gh = h @ wh.T            (fp16 matmul, fp32 PSUM accumulation)
      r  = sigmoid(gh_r + bh_r + gi_r)
      z  = sigmoid(gh_z + bh_z + gi_z)
      n  = tanh((gh_n + bh_n) * r + gi_n)
      h  = (1-z)*n + z*h                                (fp32, in place)
      fc projection of h (resp. relu(h)) -> [4, B] -> DRAM

Layouts are transposed on host so the hidden dim is the partition dim:
h^T is [H, B] = 8 chunks of [128, B]; gh^T is [3H, B] = 24 m-tiles.
Matmul inputs are fp16 (same PE rate as bf16, 3 extra mantissa bits;
weights rounded once on host, h re-rounded each step); all accumulation,
state and gate math is fp32. Measured ~9.1-9.3 ms on HW (PE ~99% busy,
clock-throttle limited), absmax rel err ~3e-4 vs the fp32 reference.
"""

import os
import sys

sys.path.insert(0, "/opt/trn_rl_repo")

import numpy as np

import concourse.mybir as mybir
import concourse.tile as tile
from concourse import bacc
from concourse.bass_utils import run_bass_kernel_spmd

N_CORES = 8
B_FULL, H, IN, OUT = 8192, 1024, 4, 4
B = B_FULL // N_CORES          # batch rows per core
P = 128                        # partitions
KT = H // P                    # h chunks (8)
GT = 3 * H // P                # gate m-tiles (24)
C = H // 2                     # encoded-context dim (512)
CT = C // P                    # ec chunks (4)
NH = 512                       # PSUM half width (bank limit for fp32)
F32 = mybir.dt.float32
BF16 = mybir.dt.float16      # matmul operand dtype
NP_MM = np.float16

_BUILD_CACHE = {}
DEBUG_DUMPS = bool(int(os.environ.get("DECODER_DEBUG", "0")))


def _build(T1, T2):
    nc = bacc.Bacc("TRN2", target_bir_lowering=False, debug=False,
                   num_devices=N_CORES)
    dbg = {}
    if DEBUG_DUMPS:
        dbg["ec"] = nc.dram_tensor("dbg_ec", [P, CT, B], BF16, kind="ExternalOutput")
        dbg["gi1"] = nc.dram_tensor("dbg_gi1", [P, GT, B], BF16, kind="ExternalOutput")
        dbg["h1"] = nc.dram_tensor("dbg_h1", [P, KT, B], F32, kind="ExternalOutput")
        for nm in ("r", "z", "n", "ghr", "prer"):
            dbg[nm] = nc.dram_tensor(f"dbg_{nm}", [P, KT, B], F32,
                                     kind="ExternalOutput")

    ctx32 = nc.dram_tensor("ctx32", [H, B], F32, kind="ExternalInput")
    ctxbf = nc.dram_tensor("ctxbf", [H, B], BF16, kind="ExternalInput")
    wcT = nc.dram_tensor("wcT", [H, C], BF16, kind="ExternalInput")
    bc = nc.dram_tensor("bc", [P, CT], F32, kind="ExternalInput")
    phases = []
    for g, T, relu_fc in ((1, T1, False), (2, T2, True)):
        wiT = nc.dram_tensor(f"wiT{g}", [C, 3 * H], BF16, kind="ExternalInput")
        whT = nc.dram_tensor(f"whT{g}", [H, 3 * H], BF16, kind="ExternalInput")
        bi = nc.dram_tensor(f"bi{g}", [P, GT], F32, kind="ExternalInput")
        bh = nc.dram_tensor(f"bh{g}", [P, GT], F32, kind="ExternalInput")
        fcT = nc.dram_tensor(f"fcT{g}", [H, 4], BF16, kind="ExternalInput")
        fcb = nc.dram_tensor(f"fcb{g}", [4, 1], F32, kind="ExternalInput")
        out_d = nc.dram_tensor(f"proj{g}", [T, 4, B], F32, kind="ExternalOutput")
        phases.append((g, T, relu_fc, wiT, whT, bi, bh, fcT, fcb, out_d))

    with tile.TileContext(nc) as tc:
        with tc.tile_pool(name="const", bufs=1) as const, \
             tc.tile_pool(name="state", bufs=1) as state, \
             tc.tile_pool(name="work", bufs=2) as work, \
             tc.tile_pool(name="projp", bufs=1) as projp, \
             tc.tile_pool(name="wij", bufs=2) as wij, \
             tc.tile_pool(name="psg", bufs=6, space="PSUM") as psg, \
             tc.tile_pool(name="psf", bufs=1, space="PSUM") as psf:

            # ---- resident constants -------------------------------------
            wc_sb = const.tile([P, KT, C], BF16, tag="wc")
            nc.sync.dma_start(out=wc_sb, in_=wcT.rearrange("(k p) c -> p k c", p=P))
            bc_sb = const.tile([P, CT], F32, tag="bc")
            nc.sync.dma_start(out=bc_sb, in_=bc[:, :])

            bias_sb = {}
            fc_sb = {}
            for (g, T, relu_fc, wiT, whT, bi, bh, fcT, fcb, out_d) in phases:
                bi_sb = const.tile([P, GT], F32, tag=f"bi{g}")
                nc.sync.dma_start(out=bi_sb, in_=bi[:, :])
                bh_sb = const.tile([P, GT], F32, tag=f"bh{g}")
                nc.sync.dma_start(out=bh_sb, in_=bh[:, :])
                f_sb = const.tile([P, KT, 4], BF16, tag=f"fc{g}")
                nc.sync.dma_start(out=f_sb, in_=fcT.rearrange("(k p) i -> p k i", p=P))
                fb_sb = const.tile([4, 1], F32, tag=f"fcb{g}")
                nc.sync.dma_start(out=fb_sb, in_=fcb[:, :])
                bias_sb[g] = (bi_sb, bh_sb)
                fc_sb[g] = (f_sb, fb_sb)

            # ---- state tiles --------------------------------------------
            h32 = state.tile([P, KT, B], F32, tag="h32")
            hbf = state.tile([P, KT, B], BF16, tag="hbf")
            rh = state.tile([P, KT, B], BF16, tag="rh")     # relu(h) for fc2
            ec_sb = state.tile([P, CT, B], BF16, tag="ec")
            gi_sb = state.tile([P, GT, B], BF16, tag="gi")
            wh_sb = state.tile([P, KT, 3 * H], BF16, tag="wh")

            nc.sync.dma_start(out=h32, in_=ctx32.rearrange("(k p) b -> p k b", p=P))
            nc.sync.dma_start(out=hbf, in_=ctxbf.rearrange("(k p) b -> p k b", p=P))

            # ---- ec = relu(wc @ ctx + bc), fp16 -------------------------
            for m in range(CT):
                for hf in range(2):
                    sl = slice(hf * NH, (hf + 1) * NH)
                    pt = psg.tile([P, NH], F32, tag="gh")
                    for k in range(KT):
                        nc.tensor.matmul(pt[:, :],
                                         wc_sb[:, k, m * P:(m + 1) * P],
                                         hbf[:, k, sl],
                                         start=(k == 0), stop=(k == KT - 1))
                    nc.scalar.activation(out=ec_sb[:, m, sl], in_=pt[:, :],
                                         func=mybir.ActivationFunctionType.Relu,
                                         bias=bc_sb[:, m:m + 1])
            if DEBUG_DUMPS:
                nc.sync.dma_start(out=dbg["ec"][:, :, :], in_=ec_sb[:, :, :])

            # ---- phases -------------------------------------------------
            for (g, T, relu_fc, wiT, whT, bi, bh, fcT, fcb, out_d) in phases:
                bi_sb, bh_sb = bias_sb[g]
                wiT_r = wiT.rearrange("(k p) t -> p k t", p=P)

                # gi = wi @ ec + bi  -> fp16, [P, GT, B]
                for m in range(GT):
                    wi_m = wij.tile([P, CT, P], BF16, tag="wim")
                    nc.sync.dma_start(out=wi_m,
                                      in_=wiT_r[:, :, m * P:(m + 1) * P])
                    for hf in range(2):
                        sl = slice(hf * NH, (hf + 1) * NH)
                        pt = psg.tile([P, NH], F32, tag="gh")
                        for k in range(CT):
                            nc.tensor.matmul(pt[:, :], wi_m[:, k, :],
                                             ec_sb[:, k, sl],
                                             start=(k == 0), stop=(k == CT - 1))
                        nc.scalar.activation(out=gi_sb[:, m, sl], in_=pt[:, :],
                                             func=mybir.ActivationFunctionType.Identity,
                                             bias=bi_sb[:, m:m + 1])

                if DEBUG_DUMPS and g == 1:
                    nc.sync.dma_start(out=dbg["gi1"][:, :, :], in_=gi_sb[:, :, :])
                # wh resident for this phase
                nc.sync.dma_start(out=wh_sb,
                                  in_=whT.rearrange("(k p) t -> p k t", p=P))
                if g == 2:
                    # restart recurrence from the context
                    nc.sync.dma_start(out=h32,
                                      in_=ctx32.rearrange("(k p) b -> p k b", p=P))
                    nc.sync.dma_start(out=hbf,
                                      in_=ctxbf.rearrange("(k p) b -> p k b", p=P))

                f_sb, fb_sb = fc_sb[g]
                for t in range(T):
                    for c in range(KT):
                        pts = []
                        for m in (c, KT + c, 2 * KT + c):   # r, z, n tiles
                            halves = []
                            for hf in range(2):
                                ph = psg.tile([P, NH], F32, tag="gh")
                                sl = slice(hf * NH, (hf + 1) * NH)
                                for k in range(KT):
                                    nc.tensor.matmul(
                                        ph[:, :],
                                        wh_sb[:, k, m * P:(m + 1) * P],
                                        hbf[:, k, sl],
                                        start=(k == 0), stop=(k == KT - 1))
                                halves.append(ph)
                            pts.append(halves)
                        pt_r, pt_z, pt_n = pts
                        A = mybir.AluOpType
                        if DEBUG_DUMPS and g == 1 and t == 0:
                            ghd = projp.tile([P, B], F32, tag="proj")
                            for hf in range(2):
                                nc.scalar.activation(
                                    out=ghd[:, hf * NH:(hf + 1) * NH],
                                    in_=pt_r[hf][:, :],
                                    func=mybir.ActivationFunctionType.Copy)
                            nc.sync.dma_start(out=dbg["ghr"][:, c, :], in_=ghd[:, :])
                        # r = sigmoid(gh_r + bh_r + gi_r); STT drains PSUM to
                        # SBUF in one pass, activation runs in place on SBUF
                        r_sb = work.tile([P, B], F32, tag="r")
                        for hf in range(2):
                            sl = slice(hf * NH, (hf + 1) * NH)
                            nc.vector.scalar_tensor_tensor(
                                out=r_sb[:, sl], in0=pt_r[hf][:, :],
                                scalar=bh_sb[:, c:c + 1], in1=gi_sb[:, c, sl],
                                op0=A.add, op1=A.add)
                        if DEBUG_DUMPS and g == 1 and t == 0:
                            pred = projp.tile([P, B], F32, tag="proj")
                            nc.scalar.activation(
                                out=pred[:, :], in_=r_sb[:, :],
                                func=mybir.ActivationFunctionType.Copy)
                            nc.sync.dma_start(out=dbg["prer"][:, c, :], in_=pred[:, :])
                        nc.scalar.activation(out=r_sb[:, :], in_=r_sb[:, :],
                                             func=mybir.ActivationFunctionType.Sigmoid)
                        # z = sigmoid(gh_z + bh_z + gi_z)
                        z_sb = work.tile([P, B], F32, tag="z")
                        for hf in range(2):
                            sl = slice(hf * NH, (hf + 1) * NH)
                            nc.vector.scalar_tensor_tensor(
                                out=z_sb[:, sl], in0=pt_z[hf][:, :],
                                scalar=bh_sb[:, KT + c:KT + c + 1],
                                in1=gi_sb[:, KT + c, sl], op0=A.add, op1=A.add)
                        nc.scalar.activation(out=z_sb[:, :], in_=z_sb[:, :],
                                             func=mybir.ActivationFunctionType.Sigmoid)
                        # n = tanh((gh_n + bh_n)*r + gi_n)
                        n_sb = work.tile([P, B], F32, tag="n")
                        for hf in range(2):
                            sl = slice(hf * NH, (hf + 1) * NH)
                            nc.vector.scalar_tensor_tensor(
                                out=n_sb[:, sl], in0=pt_n[hf][:, :],
                                scalar=bh_sb[:, 2 * KT + c:2 * KT + c + 1],
                                in1=r_sb[:, sl], op0=A.add, op1=A.mult)
                        nc.vector.tensor_add(n_sb[:, :], n_sb[:, :],
                                             gi_sb[:, 2 * KT + c, :])
                        nc.scalar.activation(out=n_sb[:, :], in_=n_sb[:, :],
                                             func=mybir.ActivationFunctionType.Tanh)
                        if DEBUG_DUMPS and g == 1 and t == 0:
                            nc.sync.dma_start(out=dbg["r"][:, c, :], in_=r_sb[:, :])
                            nc.sync.dma_start(out=dbg["z"][:, c, :], in_=z_sb[:, :])
                            nc.sync.dma_start(out=dbg["n"][:, c, :], in_=n_sb[:, :])
                        # h = (1-z)*n + z*h  ==  ((h - n) * z) + n, in place
                        hc = h32[:, c, :]
                        nc.vector.tensor_sub(hc, hc, n_sb[:, :])
                        nc.vector.tensor_mul(hc, hc, z_sb[:, :])
                        nc.vector.tensor_add(hc, hc, n_sb[:, :])
                    # fp16 copies only after ALL of this step's matmuls have
                    # read the old h (hbf is updated in place)
                    for c in range(KT):
                        # alternate engines so the 8 casts drain in parallel
                        # instead of queueing serially on ScalarE
                        if c % 2 == 0:
                            nc.scalar.activation(
                                out=hbf[:, c, :], in_=h32[:, c, :],
                                func=mybir.ActivationFunctionType.Copy)
                        else:
                            nc.vector.tensor_copy(hbf[:, c, :], h32[:, c, :])
                        if relu_fc:
                            nc.scalar.activation(out=rh[:, c, :], in_=h32[:, c, :],
                                                 func=mybir.ActivationFunctionType.Relu)
                    # fc projection of this step's h
                    src = rh if relu_fc else hbf
                    ptf = psf.tile([4, B], F32, tag="fc")
                    for hf in range(2):
                        sl = slice(hf * NH, (hf + 1) * NH)
                        for k in range(KT):
                            nc.tensor.matmul(ptf[:, sl], f_sb[:, k, :],
                                             src[:, k, sl],
                                             start=(k == 0), stop=(k == KT - 1))
                    proj = projp.tile([4, B], F32, tag="proj")
                    nc.scalar.activation(out=proj[:, :], in_=ptf[:, :],
                                         func=mybir.ActivationFunctionType.Identity,
                                         bias=fb_sb[:, 0:1])
                    nc.sync.dma_start(out=out_d[t], in_=proj[:, :])
                    if DEBUG_DUMPS and g == 1 and t == 0:
                        nc.sync.dma_start(out=dbg["h1"][:, :, :], in_=h32[:, :, :])
    nc.compile()
    return nc


def _get_nc(T1, T2):
    key = (T1, T2)
    if key not in _BUILD_CACHE:
        _BUILD_CACHE[key] = _build(T1, T2)
    return _BUILD_CACHE[key]


def kernel(context, wc, bc,
           gru1_wi, gru1_wh, gru1_bi, gru1_bh,
           gru2_wi, gru2_wh, gru2_bi, gru2_bh,
           fc_in_w, fc_in_b, fc_out_w, fc_out_b,
           future_length, past_length, _trace=False):
    T1, T2 = int(past_length), int(future_length)
    context = np.asarray(context, np.float32)

    def pT(w):     # [r, c] -> bf16 transposed [c, r]
        return np.ascontiguousarray(np.asarray(w, np.float32).T).astype(NP_MM)

    def chunked(b):   # [n*128] -> [128, n]
        v = np.asarray(b, np.float32)
        return np.ascontiguousarray(v.reshape(-1, P).T)

    shared = {
        "wcT": pT(wc), "bc": chunked(bc),
        "wiT1": pT(gru1_wi), "whT1": pT(gru1_wh),
        "bi1": chunked(gru1_bi), "bh1": chunked(gru1_bh), "fcT1": pT(fc_in_w),
        "wiT2": pT(gru2_wi), "whT2": pT(gru2_wh),
        "bi2": chunked(gru2_bi), "bh2": chunked(gru2_bh), "fcT2": pT(fc_out_w),
        "fcb1": np.asarray(fc_in_b, np.float32).reshape(4, 1).copy(),
        "fcb2": np.asarray(fc_out_b, np.float32).reshape(4, 1).copy(),
    }
    in_maps = []
    for cix in range(N_CORES):
        shard = context[cix * B:(cix + 1) * B]               # [B, H]
        ctxT = np.ascontiguousarray(shard.T)                 # [H, B]
        m = dict(shared)
        m["ctx32"] = ctxT
        m["ctxbf"] = ctxT.astype(NP_MM)
        in_maps.append(m)

    nc = _get_nc(T1, T2)
    res = run_bass_kernel_spmd(nc, in_maps, list(range(N_CORES)),
                               trace=_trace)
    kernel.last_results = res

    dec = np.empty((B_FULL, T1, IN), np.float32)
    out = np.empty((B_FULL, T2, OUT), np.float32)
    for cix in range(N_CORES):
        dec[cix * B:(cix + 1) * B] = res.results[cix]["proj1"].transpose(2, 0, 1)
        out[cix * B:(cix + 1) * B] = res.results[cix]["proj2"].transpose(2, 0, 1)
    return dec, out
